# revision 17
# baseline (speedup 1.0000x reference)
"""BinaryPathEncoder Trainium2 kernel.

Math: the reference scans depth d=0..15 applying maps @ W0.T / maps @ W1.T and
selecting per-row by path bit b_d = (u >> d) & 1 (valid while u >> d >= 2).
The final row for a value u is therefore

    v(u) = ones @ W_{b0}.T @ W_{b1}.T @ ... @ W_{b_{k-1}}.T,   k = floor(log2 u)

All v(u) for u in [2^k, 2^{k+1}) form a binary-prefix tree with the recurrence

    T_k = [ T_{k-1} @ W0.T ; T_{k-1} @ W1.T ]        (T_k row j = v(2^k + j))

which shares every prefix product exactly once (half the FLOPs of the dense
scan) and shards across 8 cores with zero communication: core c owns rows
j === c (mod 8) of every level, and the mod-8 slice obeys the *same*
recurrence (S_k = [S_{k-1} @ W0.T ; S_{k-1} @ W1.T], seeded with S_3 = v(8+c)).

Device phase 1 (SPMD x8): each core runs the 12-level slice recurrence with
the state kept transposed (X_k = S_k.T, feature dim on partitions) so every
level is plain PE matmuls (stationary = 128x128 tiles of W.T, moving = X),
and dumps each level slice to DRAM.

Host: assembles U[j] = v(unique[j]) ([4096, 512]) by fancy-indexing the dumped
tables (each needed row is one column of one core's level slice).

Device phase 2 (SPMD x8): out rows = U[mapping] via SWDGE dma_gather (2 KB row
gather from DRAM by int16 index) + contiguous DMA writes; core c produces
output rows [c*8192, (c+1)*8192).
"""

import numpy as np

try:
    import concourse  # noqa: F401
except ImportError:  # pragma: no cover
    import sys

    sys.path.insert(0, "/opt/trn_rl_repo")

import jax
import concourse.mybir as mybir
import concourse.tile as tile
from concourse import bacc, bass2jax

try:  # jax >= 0.8 moved shard_map
    from jax import shard_map
except ImportError:  # pragma: no cover
    from jax.experimental.shard_map import shard_map
from jax.sharding import Mesh, PartitionSpec

DIM = 512
P = 128
NJT = DIM // P  # 4 partition-tiles for the feature dim
N_CORES = 8
MAXK = 15  # top level: u in [2^15, 2^16)
SEEDK = 9  # host computes T_1..T_SEEDK; device runs levels SEEDK+1..15
F32 = mybir.dt.float32

# level k slice of core c: rows 2^k + 8m + c, m in [0, 2^{k-3})
def _size(k):
    return 1 << (k - 3)


def _coff(k):
    # column offset of level k inside the dumped per-core table (k > SEEDK)
    return (1 << (k - 3)) - _size(SEEDK + 1)


TOTCOL = _coff(MAXK) + _size(MAXK)  # 8064 for SEEDK=9

GATHER_CHUNK = 1024  # indices per dma_gather in phase 2
OUT_ROWS_PER_CORE = 65536 // N_CORES  # 8192


# fp32 matmul runs at 4 cycles/row on the PE; float32r (tf32-like rounded fp32)
# streams at 1 cycle/row once the moving dim is >= 256 — 4x faster for the big
# levels that dominate. The whole phase-1 chain runs in float32r (PSUM
# accumulation stays fp32; the DVE psum->sbuf copies perform the rounding).
F32R = mybir.dt.float32r


# ---------------------------------------------------------------- bass build
def _build_phase1(
    n_iters=1, dumps=True, copy_engine="vector", psum_bufs=6, dt_=None
):
    DT = F32R if dt_ is None else dt_
    nc = bacc.Bacc("TRN2", target_bir_lowering=False)
    with tile.TileContext(nc) as tc:
        with (
            tc.tile_pool(name="dram", bufs=1, space="DRAM") as dram,
            tc.tile_pool(name="consts", bufs=1) as consts,
            tc.tile_pool(name="state", bufs=1) as state,
            tc.tile_pool(name="psum", bufs=psum_bufs, space="PSUM") as psum,
        ):
            w0t_d = dram.tile([DIM, DIM], DT, kind="ExternalInput", name="w0t")
            w1t_d = dram.tile([DIM, DIM], DT, kind="ExternalInput", name="w1t")
            seed_d = dram.tile(
                [NJT, P, _size(SEEDK)], DT, kind="ExternalInput", name="seed"
            )
            table_d = dram.tile(
                [NJT, P, TOTCOL], DT, kind="ExternalOutput", name="table"
            )

            # weights: wt[w][dt] = rows [dt*128,(dt+1)*128) of W{w}.T  -> [128, 512]
            wt = [[None] * NJT for _ in range(2)]
            for w, wd in enumerate((w0t_d, w1t_d)):
                for dt in range(NJT):
                    t = consts.tile(
                        [P, DIM], DT, tag=f"wt{w}_{dt}", name=f"wt{w}_{dt}"
                    )
                    nc.sync.dma_start(t[:], wd[dt * P : (dt + 1) * P, :])
                    wt[w][dt] = t

            for it in range(n_iters):
                # per-level transposed state tiles X[k][jt] = [128, size(k)]
                X = {}
                X[SEEDK] = []
                for jt in range(NJT):
                    t = state.tile(
                        [P, _size(SEEDK)], DT, tag=f"Xs_{jt}", name=f"Xs_{jt}"
                    )
                    nc.sync.dma_start(t[:], seed_d[jt, :, :])
                    X[SEEDK].append(t)

                for k in range(SEEDK + 1, MAXK + 1):
                    rin = _size(k - 1)
                    X[k] = [
                        state.tile(
                            [P, _size(k)], DT, tag=f"X{k}_{jt}", name=f"X{k}_{jt}"
                        )
                        for jt in range(NJT)
                    ]
                    nblk = (rin + 511) // 512
                    for w in range(2):
                        for jt in range(NJT):
                            for nb in range(nblk):
                                n = min(512, rin - nb * 512)
                                ps = psum.tile([P, 512], F32, tag="ps", name="ps")
                                for dt in range(NJT):
                                    lhsT = wt[w][dt][:, jt * P : (jt + 1) * P]
                                    rhs = X[k - 1][dt][:, nb * 512 : nb * 512 + n]
                                    if n % 2 and DT == F32R:
                                        # fp32r needs an even moving dim; the
                                        # lone N=1 level falls back to fp32
                                        lhsT = lhsT.bitcast(F32)
                                        rhs = rhs.bitcast(F32)
                                    nc.tensor.matmul(
                                        ps[:, :n],
                                        lhsT,
                                        rhs,
                                        start=(dt == 0),
                                        stop=(dt == NJT - 1),
                                    )
                                dst = X[k][jt][
                                    :, w * rin + nb * 512 : w * rin + nb * 512 + n
                                ]
                                getattr(nc, copy_engine).tensor_copy(dst, ps[:, :n])
                                if dumps and k >= 14:
                                    # chunked dump so the tail isn't exposed
                                    nc.sync.dma_start(
                                        table_d[
                                            jt,
                                            :,
                                            _coff(k)
                                            + w * rin
                                            + nb * 512 : _coff(k)
                                            + w * rin
                                            + nb * 512
                                            + n,
                                        ],
                                        dst,
                                    )
                    if dumps and k < 14:
                        for jt in range(NJT):
                            nc.sync.dma_start(
                                table_d[jt, :, _coff(k) : _coff(k) + _size(k)],
                                X[k][jt][:],
                            )
    nc.compile()
    names = dict(
        w0t=w0t_d.tensor.name,
        w1t=w1t_d.tensor.name,
        seed=seed_d.tensor.name,
        table=table_d.tensor.name,
    )
    return nc, names


def _build_phase2(n_iters=1, u_dt=None, chunk=None, bufs=3):
    UDT = F32 if u_dt is None else u_dt
    cast = UDT != F32
    nch = GATHER_CHUNK if chunk is None else chunk
    nchunks = OUT_ROWS_PER_CORE // nch  # 8
    nc = bacc.Bacc("TRN2", target_bir_lowering=False)
    with tile.TileContext(nc) as tc:
        with (
            tc.tile_pool(name="dram", bufs=1, space="DRAM") as dram,
            tc.tile_pool(name="consts", bufs=1) as consts,
            tc.tile_pool(name="sb", bufs=bufs) as sb,
        ):
            u_d = dram.tile([4096, DIM], UDT, kind="ExternalInput", name="utab")
            idx_d = dram.tile(
                [P, OUT_ROWS_PER_CORE // 16], mybir.dt.int16,
                kind="ExternalInput", name="idx",
            )
            out_d = dram.tile(
                [OUT_ROWS_PER_CORE, DIM], F32, kind="ExternalOutput", name="out"
            )

            idx_sb = consts.tile(
                [P, OUT_ROWS_PER_CORE // 16], mybir.dt.int16, name="idx_sb"
            )
            nc.sync.dma_start(idx_sb[:], idx_d[:])

            for it in range(n_iters):
                for c in range(nchunks):
                    g = sb.tile([P, nch // P, DIM], UDT, tag="g", name="g")
                    nc.gpsimd.dma_gather(
                        g[:],
                        u_d[:],
                        idx_sb[:, c * (nch // 16) : (c + 1) * (nch // 16)],
                        nch,
                        nch,
                        DIM,
                    )
                    if cast:
                        gf = sb.tile([P, nch // P, DIM], F32, tag="gf", name="gf")
                        nc.vector.tensor_copy(gf[:], g[:])
                    else:
                        gf = g
                    # row j of this chunk sits at partition j%128, slot j//128
                    dview = out_d[c * nch : (c + 1) * nch, :].rearrange(
                        "(g p) e -> p g e", p=P
                    )
                    nc.sync.dma_start(dview, gf[:])
    nc.compile()
    names = dict(utab=u_d.tensor.name, idx=idx_d.tensor.name, out=out_d.tensor.name)
    return nc, names


# ---------------------------------------------------------------- runner
class SpmdRunner:
    """Reusable jitted SPMD executor for a compiled Bass module (axon/PJRT)."""

    def __init__(self, nc, n_cores=N_CORES):
        bass2jax.install_neuronx_cc_hook()
        self.nc = nc
        self.n_cores = n_cores
        self.partition_name = (
            nc.partition_id_tensor.name if nc.partition_id_tensor else None
        )
        in_names, out_names, out_avals = [], [], []
        for alloc in nc.m.functions[0].allocations:
            if not isinstance(alloc, mybir.MemoryLocationSet):
                continue
            name = alloc.memorylocations[0].name
            if alloc.kind == "ExternalInput":
                if name != self.partition_name:
                    in_names.append(name)
            elif alloc.kind == "ExternalOutput":
                out_names.append(name)
                out_avals.append(
                    jax.core.ShapedArray(
                        tuple(alloc.tensor_shape), mybir.dt.np(alloc.dtype)
                    )
                )
        self.in_names, self.out_names, self.out_avals = in_names, out_names, out_avals
        self.n_params, self.n_outs = len(in_names), len(out_avals)
        all_in = in_names + out_names + (
            [self.partition_name] if self.partition_name else []
        )

        def _bind(ins_and_bufs):
            operands = list(ins_and_bufs)
            if self.partition_name:
                operands.append(bass2jax.partition_id_tensor())
            return bass2jax._bass_exec_p.bind(
                *operands,
                out_avals=tuple(out_avals),
                in_names=tuple(all_in),
                out_names=tuple(out_names),
                lowering_input_output_aliases=(),
                sim_require_finite=True,
                sim_require_nnan=True,
                nc=nc,
            )

        def _body(*args):
            return tuple(_bind(list(args)))

        devices = jax.devices()[:n_cores]
        self.mesh = Mesh(np.asarray(devices), ("core",))
        self._body = _body
        self._jitted = None

    def _get(self):
        if self._jitted is None:
            self._jitted = jax.jit(
                shard_map(
                    self._body,
                    mesh=self.mesh,
                    in_specs=(PartitionSpec("core"),) * (self.n_params + self.n_outs),
                    out_specs=(PartitionSpec("core"),) * self.n_outs,
                    check_vma=False,
                ),
                keep_unused=True,
            )
        return self._jitted

    def run(self, in_maps, _timing=False):
        per_core = [
            [np.asarray(m[name]) for name in self.in_names] for m in in_maps
        ]
        concat_in = [
            np.concatenate([per_core[c][i] for c in range(self.n_cores)], axis=0)
            for i in range(self.n_params)
        ]
        concat_zeros = [
            np.zeros((self.n_cores * a.shape[0], *a.shape[1:]), a.dtype)
            for a in self.out_avals
        ]
        fn = self._get()
        outs = fn(*concat_in, *concat_zeros)
        jax.block_until_ready(outs)
        if _timing:
            import time

            best = float("inf")
            for _ in range(_timing if isinstance(_timing, int) and _timing > 1 else 5):
                t0 = time.perf_counter()
                outs = fn(*concat_in, *concat_zeros)
                jax.block_until_ready(outs)
                best = min(best, time.perf_counter() - t0)
            return outs, best
        return [
            {
                name: np.asarray(outs[i]).reshape(
                    self.n_cores, *self.out_avals[i].shape
                )[c]
                for i, name in enumerate(self.out_names)
            }
            for c in range(self.n_cores)
        ]


# ---------------------------------------------------------------- host logic
_CACHE = {}


def _get_programs():
    if "p1" not in _CACHE:
        nc1, names1 = _build_phase1()
        _CACHE["p1"] = (SpmdRunner(nc1), names1)
    if "p2" not in _CACHE:
        nc2, names2 = _build_phase2()
        _CACHE["p2"] = (SpmdRunner(nc2), names2)
    return _CACHE["p1"], _CACHE["p2"]


def _host_tables(w0t, w1t):
    """T_0..T_SEEDK on host (fp32): ~0.5 GFLOP of numpy, 1.5%% of the work."""
    tabs = [np.ones((1, DIM), np.float32)]
    for k in range(1, SEEDK + 1):
        tabs.append(
            np.concatenate([tabs[k - 1] @ w0t, tabs[k - 1] @ w1t], axis=0)
        )
    return tabs


def _wrap_idx16(idx):
    # dma_gather index layout: index j at partition j%16, slot j//16,
    # replicated across the 8 gpsimd cores (8 x 16 partitions)
    n = idx.shape[0]
    w = idx.astype(np.int16).reshape(n // 16, 16).T  # [16, n//16]
    return np.tile(w, (8, 1)).copy()  # [128, n//16]


def kernel(unique, mapping, primitives):
    unique = np.asarray(unique).astype(np.int64)
    mapping = np.asarray(mapping).astype(np.int64)
    primitives = np.asarray(primitives).astype(np.float32)
    out_dt = np.asarray(primitives).dtype

    w0t = np.ascontiguousarray(primitives[0].T)
    w1t = np.ascontiguousarray(primitives[1].T)

    (run1, names1), (run2, names2) = _get_programs()

    # host head: T_0..T_9; per-core seeds = mod-8 slice of T_9, transposed
    htabs = _host_tables(w0t, w1t)
    seeds = [
        np.ascontiguousarray(htabs[SEEDK][c::8].T).reshape(NJT, P, _size(SEEDK))
        for c in range(N_CORES)
    ]
    in1 = [
        {names1["w0t"]: w0t, names1["w1t"]: w1t, names1["seed"]: seeds[c]}
        for c in range(N_CORES)
    ]
    res1 = run1.run(in1)
    tabs = np.stack([res1[c][names1["table"]] for c in range(N_CORES)])  # [8,4,128,TOT]

    # host: U[j] = v(unique[j])
    u = unique
    k = np.zeros_like(u)
    nz = u >= 2
    k[nz] = np.floor(np.log2(u[nz].astype(np.float64))).astype(np.int64)
    U = np.empty((u.shape[0], DIM), np.float32)
    hi = k > SEEDK  # rows that live in the device tables
    r = u[hi] - (1 << k[hi])
    core = r & 7
    col = (1 << (k[hi] - 3)) - _size(SEEDK + 1) + (r >> 3)
    U[hi] = tabs[core, :, :, col].reshape(-1, DIM)
    lo = np.nonzero(~hi)[0]
    for j in lo:
        uj = int(u[j])
        U[j] = 1.0 if uj < 2 else htabs[int(k[j])][uj - (1 << int(k[j]))]

    # phase 2: out rows = U[mapping]
    flat = mapping.reshape(-1)
    in2 = []
    for c in range(N_CORES):
        shard = flat[c * OUT_ROWS_PER_CORE : (c + 1) * OUT_ROWS_PER_CORE]
        in2.append(
            {names2["utab"]: U, names2["idx"]: _wrap_idx16(shard)}
        )
    res2 = run2.run(in2)
    out = np.concatenate(
        [res2[c][names2["out"]] for c in range(N_CORES)], axis=0
    ).reshape(*mapping.shape, DIM)
    return out.astype(out_dt, copy=False)
